# revision 36
# baseline (speedup 1.0000x reference)
"""Trainium2 Bass kernel for nn_BlockDrop (Swin-style transformer block).

Reference math (per batch image):
  h = LN1(x); 16x16 windows of 256 tokens; 16-head attention (d=64) with
  separate Q/K/V/O linears; x += attn; h2 = LN2(x); x += W2@gelu(W1@h2).

Sharding: pure data parallel — batch image b -> core b (16 windows each,
no cross-core communication). Host performs window reordering,
transposition (feature-major) and weight folding.

v2: fp8e4 DoubleRow matmuls (2x PE throughput) for QKV, Wo, attn@V and
W2; scores and W1 stay bf16 for the error budget. fp8 weights fit all
six weight sets in SBUF simultaneously, so the kernel runs ONE fused
pass per 512-token tile with no DRAM round-trips for intermediates.
Software pipelining: the MLP of tile i-1 is emitted between tile i's
LN1 stats and its attention, hiding LN post-processing latency.
LayerNorm stats via ones-matmuls; LN2 mean reuses LN1's sum plus a
rank-1 colsum matmul over the attention output (no f32 sum matmul).
Softmax: scores^T, exp scaled by 1/16 into fp8 (denominator via a ones
column in V, cancels exactly); 1/s broadcast by selector matmuls.
"""
import numpy as np
import ml_dtypes

import concourse.bass as bass
import concourse.mybir as mybir
import concourse.tile as tile
from concourse.bass_utils import run_bass_kernel_spmd

f32 = mybir.dt.float32
bf16 = mybir.dt.bfloat16
f8 = mybir.dt.float8e4
AF = mybir.ActivationFunctionType
ALU = mybir.AluOpType
DR = mybir.MatmulPerfMode.DoubleRow

DIM = 1024
HEADS = 16
HDIM = 64
HID = 4096
SCALE = HDIM ** -0.5
EPS = 1e-5
T = 4096          # tokens per core
TT = 512          # tokens per T-tile (2 windows)
NC = 8            # 128-ch chunks of DIM
NP = 4            # 256-ch pairs of DIM
WS2 = 256         # tokens per window

SQ = 512.0        # fp8 scale for Wq (includes 1/8 attention scale)
SW = 64.0         # fp8 scale for Wk/Wv/Wo/W2
SCS = 16.0        # fp8 scale for wo colsum
NEG_LN16 = -2.7725887  # exp bias so e' = exp(s)/16 stays in fp8 range


def _split_multi_waits(nc):
    """This walrus rejects >1 sync-wait per instruction. Move extra waits
    onto same-engine NoOps inserted just before (engine queues are FIFO,
    so blocking the queue on each sem in turn is equivalent)."""
    n_split = 0
    for fn in nc.m.functions:
        for blk in fn.blocks:
            insts = blk.instructions
            new = []
            for inst in insts:
                si = inst.sync_info
                waits = list(si.on_wait) if si is not None else []
                if len(waits) > 1:
                    for w in waits[:-1]:
                        n_split += 1
                        new.append(mybir.InstNoOp(
                            name=f"{inst.name}-ws{n_split}",
                            engine=inst.engine, ins=[], outs=[],
                            sync_info=mybir.SyncInfo(on_wait=[w], on_update=[]),
                        ))
                    inst.sync_info = mybir.SyncInfo(
                        on_wait=[waits[-1]], on_update=list(si.on_update))
                new.append(inst)
            if len(new) != len(insts):
                blk.instructions[:] = new
    return n_split


def build_nc(NT=8, use_f32r=False, xin_bufs=2):
    nc = bass.Bass()

    xT_e = nc.declare_dram_parameter("xT", [DIM, T], bf16, isOutput=False)
    wqp_e = nc.declare_dram_parameter("wqp", [128, 2 * NP, DIM], f8, isOutput=False)
    wkp_e = nc.declare_dram_parameter("wkp", [128, 2 * NP, DIM], f8, isOutput=False)
    wvp_e = nc.declare_dram_parameter("wvp", [128, 2 * NP, DIM], f8, isOutput=False)
    wop_e = nc.declare_dram_parameter("wop", [128, 2 * NP, DIM], f8, isOutput=False)
    w1_e = nc.declare_dram_parameter("w1", [DIM, HID], bf16, isOutput=False)
    w2p_e = nc.declare_dram_parameter("w2p", [128, 32, DIM], f8, isOutput=False)
    wocs_e = nc.declare_dram_parameter("wocs", [128, 2 * NP, 1], f8, isOutput=False)
    bqk_e = nc.declare_dram_parameter("bqk", [128, 16], f32, isOutput=False)
    boc_e = nc.declare_dram_parameter("boc", [128, NC], f32, isOutput=False)
    b1c_e = nc.declare_dram_parameter("b1c", [128, 32], f32, isOutput=False)
    b2c_e = nc.declare_dram_parameter("b2c", [128, NC], f32, isOutput=False)
    sbo_e = nc.declare_dram_parameter("sbo", [1, 1], f32, isOutput=False)
    sel_e = nc.declare_dram_parameter("sel", [128, 256], bf16, isOutput=False)
    yT_e = nc.declare_dram_parameter("yT", [DIM, T], bf16, isOutput=True)

    with tile.TileContext(nc) as tc:
        with (
            tc.tile_pool(name="wt", bufs=1) as wt,
            tc.tile_pool(name="cst", bufs=1) as cst,
            tc.tile_pool(name="act", bufs=1) as act,
            tc.tile_pool(name="psA", bufs=8, space="PSUM") as psA,
        ):
            # ---- constants ----
            bqk = cst.tile([128, 16], f32)
            boc = cst.tile([128, NC], f32)
            b1c = cst.tile([128, 32], f32)
            b2c = cst.tile([128, NC], f32)
            sbo = cst.tile([1, 1], f32)
            sel = cst.tile([128, 256], bf16)
            wocs = cst.tile([128, 2 * NP, 1], f8)
            for dst, srcp in ((bqk, bqk_e), (boc, boc_e), (b1c, b1c_e),
                              (b2c, b2c_e), (sbo, sbo_e), (sel, sel_e),
                              (wocs, wocs_e)):
                nc.sync.dma_start(out=dst, in_=srcp[:])
            ones_q = cst.tile([128, 1], bf16)    # LN sum/sumsq lhsT
            ones_b = cst.tile([1, 128], bf16)    # K=1 broadcast lhsT
            eps_t = cst.tile([1, 1], f32)
            nl16 = cst.tile([128, 1], f32)
            nc.vector.memset(ones_q, 1.0)
            nc.vector.memset(ones_b, 1.0)
            nc.vector.memset(eps_t, EPS)
            nc.vector.memset(nl16, NEG_LN16)

            # per-iteration state handed to the delayed MLP section
            state = {}

            def emit_x_load(it):
                t0 = it * TT
                xt = [act.tile([128, TT], bf16, name=f"xt{c}", tag=f"xt{c}",
                               bufs=xin_bufs) for c in range(NC)]
                for c in range(NC):
                    nc.sync.dma_start(out=xt[c], in_=xT_e[c * 128:(c + 1) * 128,
                                                          t0:t0 + TT])
                return xt

            def emit_ln1_stats(xt):
                ps_s = psA.tile([1, TT], f32, name="ps_s1", tag="psA")
                ps_q = psA.tile([1, TT], f32, name="ps_q1", tag="psA")
                sqs = []
                for c in range(NC):
                    sq = act.tile([128, TT], bf16, name="sq1", tag="sq", bufs=2)
                    nc.vector.tensor_mul(sq, xt[c], xt[c])
                    sqs.append(sq)
                    nc.tensor.matmul(ps_s, lhsT=ones_q, rhs=xt[c],
                                     start=(c == 0), stop=(c == NC - 1))
                for c in range(NC):
                    nc.tensor.matmul(ps_q, lhsT=ones_q, rhs=sqs[c],
                                     start=(c == 0), stop=(c == NC - 1))
                m1b = act.tile([1, TT], bf16, name="m1b", tag="m1b")
                nc.scalar.activation(m1b, ps_s, AF.Copy, scale=1.0 / DIM)
                exq = act.tile([1, TT], bf16, name="exq1", tag="exq", bufs=2)
                nc.scalar.activation(exq, ps_q, AF.Copy, scale=1.0 / DIM)
                msq = act.tile([1, TT], bf16, name="msq1", tag="msq")
                nc.vector.tensor_mul(msq, m1b, m1b)
                nc.vector.tensor_sub(exq, exq, msq)
                nc.scalar.activation(exq, exq, AF.Ln, bias=eps_t)
                rs1 = act.tile([1, TT], bf16, name="rs1", tag="rs1")
                nc.scalar.activation(rs1, exq, AF.Exp, scale=-0.5)
                return m1b, rs1

            def emit_ln_bcast(mrow, rrow, nm):
                """Broadcast the per-token mean/rstd rows to all 128
                partitions as bf16 SBUF tiles (ones-matmul + Scalar copy),
                so the DVE apply ops run in the 2x all-16-bit mode."""
                mbc = act.tile([128, TT], bf16, name=f"mbc{nm}", tag="scb", bufs=4)
                rbc = act.tile([128, TT], bf16, name=f"rbc{nm}", tag="scb", bufs=4)
                for row, bc in ((mrow, mbc), (rrow, rbc)):
                    ps = psA.tile([128, TT], f32, name=f"ps_bc{nm}", tag="psA")
                    nc.tensor.matmul(ps, lhsT=ones_b, rhs=row, start=True, stop=True)
                    nc.scalar.activation(bc, ps, AF.Copy)
                return mbc, rbc

            def emit_ln1_apply(xt, m1b, rs1):
                mbc, rbc = emit_ln_bcast(m1b, rs1, "1")
                hp = act.tile([128, NC, TT], f8, name="hp", tag="p8")
                for c in range(NC):
                    cen = act.tile([128, TT], bf16, name="cen1", tag="cen", bufs=2)
                    nc.vector.tensor_sub(cen, xt[c], mbc)
                    nc.vector.tensor_mul(hp[:, c, :], cen, rbc)
                return hp

            def emit_mlp_ln(it):
                """LN2-apply for tile `it` (stats/r from emit_attn(it))."""
                st = state
                r_sb, m2row, r2row = st["r"], st["m2row"], st["r2row"]
                mbc, rbc = emit_ln_bcast(m2row, r2row, "2")
                h2b = []
                for c in range(NC):
                    cen = act.tile([128, TT], bf16, name="cen", tag="cen", bufs=2)
                    nc.vector.tensor_sub(cen, r_sb[c], mbc)
                    h2 = act.tile([128, TT], bf16, name=f"h2_{c}", tag=f"oh{c}")
                    nc.vector.tensor_mul(h2, cen, rbc)
                    h2b.append(h2)
                st["h2b"] = h2b

            def emit_mlp_mm(it):
                """W1 + gelu + W2 + residual for tile `it`."""
                t0 = it * TT
                st = state
                r_sb, h2b = st["r"], st["h2b"]
                g_sb = []
                for j in range(16):
                    g_sb.append(act.tile([128, 2, TT], f8, name=f"g{j}",
                                         tag=f"qkg{j}"))
                for hj in range(32):
                    ps = psA.tile([128, TT], f32, name="ps_w1", tag="psA")
                    for c in range(NC):
                        nc.tensor.matmul(ps, lhsT=w1_sb[c][:, hj * 128:(hj + 1) * 128],
                                         rhs=h2b[c], start=(c == 0), stop=(c == NC - 1))
                    nc.scalar.activation(g_sb[hj // 2][:, hj % 2, :], ps, AF.Gelu,
                                         bias=b1c[:, hj:hj + 1])
                for co in range(NC):
                    ps = psA.tile([128, TT], f32, name="ps_w2", tag="psA")
                    for j in range(16):
                        nc.tensor.matmul(ps, lhsT=w2_sb[j][:, :, co * 128:(co + 1) * 128],
                                         rhs=g_sb[j], perf_mode=DR,
                                         start=(j == 0), stop=(j == 15))
                    mo = act.tile([128, TT], bf16, name="mo", tag="tmp", bufs=2)
                    nc.scalar.activation(mo, ps, AF.Identity, scale=1.0 / SW,
                                         bias=b2c[:, co:co + 1])
                    nc.gpsimd.tensor_add(r_sb[co], r_sb[co], mo)
                    nc.sync.dma_start(out=yT_e[co * 128:(co + 1) * 128, t0:t0 + TT],
                                      in_=r_sb[co])

            def emit_attn(it, xt, hp):
                """QKV + attention + Wo + LN2 stats for tile it."""
                # ---- QKV ----
                q_sb = [act.tile([128, TT], bf16, name=f"q{c}", tag=f"qkg{c}")
                        for c in range(NC)]
                k_sb = [act.tile([128, TT], bf16, name=f"k{c}", tag=f"qkg{8 + c}")
                        for c in range(NC)]
                for co in range(NC):
                    ps = psA.tile([128, TT], f32, name="ps_q", tag="psA")
                    for j in range(NP):
                        nc.tensor.matmul(ps, lhsT=wq_sb[j][:, :, co * 128:(co + 1) * 128],
                                         rhs=hp[:, 2 * j:2 * j + 2, :], perf_mode=DR,
                                         start=(j == 0), stop=(j == NP - 1))
                    nc.vector.tensor_scalar(q_sb[co], ps, 1.0 / SQ,
                                            bqk[:, co:co + 1], ALU.mult, ALU.add)
                    ps = psA.tile([128, TT], f32, name="ps_k", tag="psA")
                    for j in range(NP):
                        nc.tensor.matmul(ps, lhsT=wk_sb[j][:, :, co * 128:(co + 1) * 128],
                                         rhs=hp[:, 2 * j:2 * j + 2, :], perf_mode=DR,
                                         start=(j == 0), stop=(j == NP - 1))
                    nc.vector.tensor_scalar(k_sb[co], ps, 1.0 / SW,
                                            bqk[:, 8 + co:8 + co + 1], ALU.mult, ALU.add)
                v_sb = [act.tile([128, HEADS, 2, 80], f8, name=f"v{w}", tag=f"v{w}")
                        for w in range(2)]
                for tc_ in range(4):
                    w, i = tc_ // 2, tc_ % 2
                    for nh in range(2):
                        ps = psA.tile([128, TT], f32, name="ps_v", tag="psA")
                        for j in range(NP):
                            nc.tensor.matmul(
                                ps, lhsT=hp[:, 2 * j:2 * j + 2, tc_ * 128:(tc_ + 1) * 128],
                                rhs=wv_sb[j][:, :, nh * 512:(nh + 1) * 512],
                                perf_mode=DR, start=(j == 0), stop=(j == NP - 1))
                        nc.vector.tensor_scalar_mul(
                            v_sb[w][:, nh * 8:(nh + 1) * 8, i, 0:64],
                            ps.rearrange("p (h d) -> p h d", d=64), 1.0 / SW)
                for w in range(2):
                    nc.vector.memset(v_sb[w][:, :, :, 64:65], 1.0)

                # ---- attention: 4-head groups, o lags one group ----
                sc = [act.tile([128, TT], bf16, name=f"sc{g}", tag="scb", bufs=4)
                      for g in range(4)]
                for g in range(4):
                    nc.vector.memset(sc[g], 1.0)
                oT = [act.tile([128, TT], bf16, name=f"oT{c}", tag=f"oh{c}")
                      for c in range(NC)]

                def emit_o(w, h0, e_g):
                    ws = w * WS2
                    for k2 in range(2):
                        hpair = (h0 + 2 * k2, h0 + 2 * k2 + 1)
                        ps_o = psA.tile([65, TT], f32, name="ps_o", tag="psA")
                        for j, h in enumerate(hpair):
                            nc.tensor.matmul(
                                ps_o[:, j * WS2:(j + 1) * WS2],
                                lhsT=v_sb[w][:, h, :, 0:65],
                                rhs=e_g[h].rearrange("p (two n) -> p two n", two=2),
                                perf_mode=DR, start=(j == 0), stop=(j == 1))
                        for j, h in enumerate(hpair):
                            ch, hh = h // 2, 64 * (h % 2)
                            nc.vector.tensor_copy(
                                sc[h // 4][32 * (h % 4):32 * (h % 4) + 1, ws:ws + WS2],
                                ps_o[64:65, j * WS2:(j + 1) * WS2])
                            nc.any.tensor_copy(oT[ch][hh:hh + 64, ws:ws + WS2],
                                               ps_o[0:64, j * WS2:(j + 1) * WS2])

                pend = None
                for w in range(2):
                    ws = w * WS2
                    for h0 in range(0, HEADS, 4):
                        e_g = {}
                        ps_s_g = {}
                        for h in range(h0, h0 + 4):
                            ch, hh = h // 2, 64 * (h % 2)
                            ps_s = psA.tile([128, TT], f32, name="ps_sT", tag="psA")
                            nc.tensor.matmul(ps_s[:, 0:WS2],
                                             lhsT=k_sb[ch][hh:hh + 64, ws:ws + 128],
                                             rhs=q_sb[ch][hh:hh + 64, ws:ws + WS2],
                                             start=True, stop=False)
                            nc.tensor.matmul(ps_s[:, WS2:TT],
                                             lhsT=k_sb[ch][hh:hh + 64, ws + 128:ws + WS2],
                                             rhs=q_sb[ch][hh:hh + 64, ws:ws + WS2],
                                             start=False, stop=True)
                            ps_s_g[h] = ps_s
                        if pend is not None:
                            emit_o(*pend)
                        for h in range(h0, h0 + 4):
                            e_sb = act.tile([128, TT], f8, name="e_sb", tag="e", bufs=5)
                            nc.scalar.activation(e_sb, ps_s_g[h], AF.Exp, bias=nl16)
                            e_g[h] = e_sb
                        pend = (w, h0, e_g)
                emit_o(*pend)

                # ---- 1/s + normalize -> fp8 pairs ----
                with nc.allow_low_precision(reason="1/s as bf16 matmul operand"):
                    for g in range(4):
                        nc.scalar.activation(sc[g], sc[g], AF.Ln)
                        nc.scalar.activation(sc[g], sc[g], AF.Exp, scale=-1.0)
                oTp = act.tile([128, NC, TT], f8, name="oTp", tag="p8")
                for j in range(NC):
                    ps_b = psA.tile([128, TT], f32, name="ps_rsb", tag="psA")
                    nc.tensor.matmul(ps_b, lhsT=sel[:, 128 * (j % 2):128 * (j % 2) + 128],
                                     rhs=sc[j // 2], start=True, stop=True)
                    nc.vector.tensor_mul(oTp[:, j, :], oT[j], ps_b)

                # ---- LN2 mean (colsum over oTp; reuses LN1's sum) ----
                ps_s2 = psA.tile([1, TT], f32, name="ps_s2", tag="psA")
                for c in range(NC):
                    nc.tensor.matmul(ps_s2, lhsT=wocs[:, c, :], rhs=oTp[:, c, :],
                                     start=(c == 0), stop=(c == NC - 1))
                # m2 = m1 + sum_o/(SCS*DIM) + sum_bo/DIM
                m2f = act.tile([1, TT], bf16, name="m2f", tag="exq", bufs=2)
                nc.vector.tensor_scalar(m2f, ps_s2, 1.0 / (SCS * DIM),
                                        sbo, ALU.mult, ALU.add)
                m2row = act.tile([1, TT], bf16, name="m2row", tag="m2row", bufs=2)
                nc.vector.tensor_add(m2row, m2f, m1b)

                # ---- Wo + residual (LN2 sumsq pipelined per chunk) ----
                r_sb = [act.tile([128, TT], bf16, name=f"r{c}", tag=f"r{c}")
                        for c in range(NC)]
                ps_q2 = psA.tile([1, TT], f32, name="ps_q2", tag="psA")
                for co in range(NC):
                    ps = psA.tile([128, TT], f32, name="ps_wo", tag="psA")
                    for j in range(NP):
                        nc.tensor.matmul(ps, lhsT=wo_sb[j][:, :, co * 128:(co + 1) * 128],
                                         rhs=oTp[:, 2 * j:2 * j + 2, :], perf_mode=DR,
                                         start=(j == 0), stop=(j == NP - 1))
                    wos = act.tile([128, TT], bf16, name="wos", tag="tmp", bufs=2)
                    nc.scalar.activation(wos, ps, AF.Identity, scale=1.0 / SW,
                                         bias=boc[:, co:co + 1])
                    nc.vector.tensor_add(r_sb[co], xt[co], wos)
                    sq = act.tile([128, TT], bf16, name="sq2", tag="sq", bufs=2)
                    nc.vector.tensor_mul(sq, r_sb[co], r_sb[co])
                    nc.tensor.matmul(ps_q2, lhsT=ones_q, rhs=sq,
                                     start=(co == 0), stop=(co == NC - 1))
                exq = act.tile([1, TT], bf16, name="exq2", tag="exq", bufs=2)
                nc.scalar.activation(exq, ps_q2, AF.Copy, scale=1.0 / DIM)
                msq = act.tile([1, TT], bf16, name="msq2", tag="msq")
                nc.vector.tensor_mul(msq, m2row, m2row)
                nc.vector.tensor_sub(exq, exq, msq)
                nc.scalar.activation(exq, exq, AF.Ln, bias=eps_t)
                r2row = act.tile([1, TT], bf16, name="r2row", tag="r2row", bufs=2)
                nc.scalar.activation(r2row, exq, AF.Exp, scale=-0.5)
                state.update(r=r_sb, m2row=m2row, r2row=r2row)

            # ================= fused, software-pipelined pass =============
            xt_cur = emit_x_load(0)

            # ---- resident weights (after x(0) so tile 0 starts early) ----
            wq_sb, wk_sb, wv_sb, wo_sb = [], [], [], []
            for lst, src, nm in ((wq_sb, wqp_e, "wq"), (wk_sb, wkp_e, "wk"),
                                 (wv_sb, wvp_e, "wv"), (wo_sb, wop_e, "wo")):
                for j in range(NP):
                    t_ = wt.tile([128, 2, DIM], f8, name=f"{nm}{j}")
                    nc.sync.dma_start(out=t_, in_=src[:, 2 * j:2 * j + 2, :])
                    lst.append(t_)
            w1_sb = []
            for c in range(NC):
                t_ = wt.tile([128, HID], bf16, name=f"w1_{c}")
                nc.sync.dma_start(out=t_, in_=w1_e[c * 128:(c + 1) * 128, :])
                w1_sb.append(t_)
            w2_sb = []
            for j in range(16):
                t_ = wt.tile([128, 2, DIM], f8, name=f"w2_{j}")
                nc.sync.dma_start(out=t_, in_=w2p_e[:, 2 * j:2 * j + 2, :])
                w2_sb.append(t_)

            for it in range(NT):
                xt_next = emit_x_load(it + 1) if it + 1 < NT else None
                m1b, rs1 = emit_ln1_stats(xt_cur)
                if it > 0:
                    emit_mlp_ln(it - 1)
                hp = emit_ln1_apply(xt_cur, m1b, rs1)
                if it > 0:
                    emit_mlp_mm(it - 1)
                emit_attn(it, xt_cur, hp)
                xt_cur = xt_next
            emit_mlp_ln(NT - 1)
            emit_mlp_mm(NT - 1)

    _split_multi_waits(nc)
    return nc


# ---------------------------------------------------------------------------
# Host side
# ---------------------------------------------------------------------------
_CACHE = {}


def _bf(a):
    return np.ascontiguousarray(a).astype(ml_dtypes.bfloat16)


def _q8(w, s):
    """Quantize to TRN fp8e4 (max 240) with scale s, packed as 128-row pairs:
    out[r, 2j+i, c] = fp8(s * w[256j + 128i + r, c])."""
    q = np.clip(w * s, -240.0, 240.0).astype(ml_dtypes.float8_e4m3)
    K = w.shape[0]
    return np.ascontiguousarray(
        q.reshape(K // 128, 128, -1).transpose(1, 0, 2))


def prep_consts(g1, beta1, Wq, bq, Wk, bk, Wv, bv, Wo, bo, g2, beta2,
                W1, b1m, W2, b2m):
    Wq_e = (g1[:, None] * Wq) * SCALE
    bq_e = (beta1 @ Wq + bq) * SCALE
    Wk_e = g1[:, None] * Wk
    bk_e = beta1 @ Wk + bk
    Wv_e = g1[:, None] * Wv
    bv_e = beta1 @ Wv + bv
    bo_e = bv_e @ Wo + bo
    W1_e = g2[:, None] * W1
    b1_e = beta2 @ W1 + b1m
    bqk = np.concatenate([bq_e.reshape(8, 128).T, bk_e.reshape(8, 128).T], axis=1)
    sel = np.zeros((128, 256), np.float32)
    sel[0, 0:64] = 1.0
    sel[32, 64:128] = 1.0
    sel[64, 128 + 0:128 + 64] = 1.0
    sel[96, 128 + 64:128 + 128] = 1.0
    wop = _q8(Wo, SW)
    # colsum of the QUANTIZED Wo (so LN2's mean matches the computed r)
    wo_deq = wop.astype(np.float32).transpose(1, 0, 2).reshape(DIM, DIM) / SW
    wocs = np.clip(wo_deq.sum(axis=1) * SCS, -240, 240).astype(ml_dtypes.float8_e4m3)
    wocs = np.ascontiguousarray(
        wocs.reshape(NP * 2, 128).T.reshape(128, NP * 2, 1))
    return {
        "wqp": _q8(Wq_e, SQ), "wkp": _q8(Wk_e, SW), "wvp": _q8(Wv_e, SW),
        "wop": wop, "wocs": wocs,
        "w1": _bf(W1_e), "w2p": _q8(W2, SW),
        "bqk": np.ascontiguousarray(bqk.astype(np.float32)),
        "boc": np.ascontiguousarray(bo_e.reshape(NC, 128).T.astype(np.float32)),
        "b1c": np.ascontiguousarray(b1_e.reshape(32, 128).T.astype(np.float32)),
        "b2c": np.ascontiguousarray(b2m.reshape(NC, 128).T.astype(np.float32)),
        "sbo": np.array([[bo_e.sum() / DIM]], np.float32),
        "sel": _bf(sel),
    }


def window_order(x_b):
    # [4096, C] row-major spatial -> window-contiguous [4096, C]
    C = x_b.shape[-1]
    t = x_b.reshape(4, 16, 4, 16, C).transpose(0, 2, 1, 3, 4)
    return t.reshape(4096, C)


def window_unorder(y_b):
    C = y_b.shape[-1]
    t = y_b.reshape(4, 4, 16, 16, C).transpose(0, 2, 1, 3, 4)
    return t.reshape(4096, C)


def kernel(x, g1, beta1, Wq, bq, Wk, bk, Wv, bv, Wo, bo, g2, beta2,
           W1, b1m, W2, b2m, window_size, spatial_h, spatial_w):
    x = np.asarray(x, np.float32)
    args = [np.asarray(a, np.float32) for a in
            (g1, beta1, Wq, bq, Wk, bk, Wv, bv, Wo, bo, g2, beta2, W1, b1m, W2, b2m)]
    consts = prep_consts(*args)

    if "nc" not in _CACHE:
        _CACHE["nc"] = build_nc(NT=8)
    nc = _CACHE["nc"]

    B = x.shape[0]
    in_maps = []
    for c in range(B):
        xw = window_order(x[c])                       # [4096, C]
        m = {"xT": np.ascontiguousarray(xw.T).astype(ml_dtypes.bfloat16)}
        m.update(consts)
        in_maps.append(m)
    res = run_bass_kernel_spmd(nc, in_maps, core_ids=list(range(B)))
    out = np.empty_like(x)
    for c in range(B):
        yT = res.results[c]["yT"].astype(np.float32)  # [C, 4096]
        out[c] = window_unorder(np.ascontiguousarray(yT.T))
    return out


# revision 37
# speedup vs baseline: 1.0521x; 1.0521x over previous
"""Trainium2 Bass kernel for nn_BlockDrop (Swin-style transformer block).

Reference math (per batch image):
  h = LN1(x); 16x16 windows of 256 tokens; 16-head attention (d=64) with
  separate Q/K/V/O linears; x += attn; h2 = LN2(x); x += W2@gelu(W1@h2).

Sharding: pure data parallel — batch image b -> core b (16 windows each,
no cross-core communication). Host performs window reordering,
transposition (feature-major) and weight folding.

v2: fp8e4 DoubleRow matmuls (2x PE throughput) for QKV, Wo, attn@V and
W2; scores and W1 stay bf16 for the error budget. fp8 weights fit all
six weight sets in SBUF simultaneously, so the kernel runs ONE fused
pass per 512-token tile with no DRAM round-trips for intermediates.
Software pipelining: the MLP of tile i-1 is emitted between tile i's
LN1 stats and its attention, hiding LN post-processing latency.
LayerNorm stats via ones-matmuls; LN2 mean reuses LN1's sum plus a
rank-1 colsum matmul over the attention output (no f32 sum matmul).
Softmax: scores^T, exp scaled by 1/16 into fp8 (denominator via a ones
column in V, cancels exactly); 1/s broadcast by selector matmuls.
"""
import numpy as np
import ml_dtypes

import concourse.bass as bass
import concourse.mybir as mybir
import concourse.tile as tile
from concourse.bass_utils import run_bass_kernel_spmd

f32 = mybir.dt.float32
bf16 = mybir.dt.bfloat16
f8 = mybir.dt.float8e4
AF = mybir.ActivationFunctionType
ALU = mybir.AluOpType
DR = mybir.MatmulPerfMode.DoubleRow

DIM = 1024
HEADS = 16
HDIM = 64
HID = 4096
SCALE = HDIM ** -0.5
EPS = 1e-5
T = 4096          # tokens per core
TT = 512          # tokens per T-tile (2 windows)
NC = 8            # 128-ch chunks of DIM
NP = 4            # 256-ch pairs of DIM
WS2 = 256         # tokens per window

SQ = 512.0        # fp8 scale for Wq (includes 1/8 attention scale)
SW = 64.0         # fp8 scale for Wk/Wv/Wo/W2
SCS = 16.0        # fp8 scale for wo colsum
NEG_LN16 = -2.7725887  # exp bias so e' = exp(s)/16 stays in fp8 range


def _split_multi_waits(nc):
    """This walrus rejects >1 sync-wait per instruction. Move extra waits
    onto same-engine NoOps inserted just before (engine queues are FIFO,
    so blocking the queue on each sem in turn is equivalent)."""
    n_split = 0
    for fn in nc.m.functions:
        for blk in fn.blocks:
            insts = blk.instructions
            new = []
            for inst in insts:
                si = inst.sync_info
                waits = list(si.on_wait) if si is not None else []
                if len(waits) > 1:
                    for w in waits[:-1]:
                        n_split += 1
                        new.append(mybir.InstNoOp(
                            name=f"{inst.name}-ws{n_split}",
                            engine=inst.engine, ins=[], outs=[],
                            sync_info=mybir.SyncInfo(on_wait=[w], on_update=[]),
                        ))
                    inst.sync_info = mybir.SyncInfo(
                        on_wait=[waits[-1]], on_update=list(si.on_update))
                new.append(inst)
            if len(new) != len(insts):
                blk.instructions[:] = new
    return n_split


def build_nc(NT=8, use_f32r=False, xin_bufs=1):
    nc = bass.Bass()

    xT_e = nc.declare_dram_parameter("xT", [DIM, T], bf16, isOutput=False)
    wqp_e = nc.declare_dram_parameter("wqp", [128, 2 * NP, DIM], f8, isOutput=False)
    wkp_e = nc.declare_dram_parameter("wkp", [128, 2 * NP, DIM], f8, isOutput=False)
    wvp_e = nc.declare_dram_parameter("wvp", [128, 2 * NP, DIM], f8, isOutput=False)
    wop_e = nc.declare_dram_parameter("wop", [128, 2 * NP, DIM], f8, isOutput=False)
    w1_e = nc.declare_dram_parameter("w1", [DIM, HID], bf16, isOutput=False)
    w2p_e = nc.declare_dram_parameter("w2p", [128, 32, DIM], f8, isOutput=False)
    wocs_e = nc.declare_dram_parameter("wocs", [128, 2 * NP, 1], f8, isOutput=False)
    bqk_e = nc.declare_dram_parameter("bqk", [128, 16], f32, isOutput=False)
    boc_e = nc.declare_dram_parameter("boc", [128, NC], f32, isOutput=False)
    b1c_e = nc.declare_dram_parameter("b1c", [128, 32], f32, isOutput=False)
    b2c_e = nc.declare_dram_parameter("b2c", [128, NC], f32, isOutput=False)
    sbo_e = nc.declare_dram_parameter("sbo", [1, 1], f32, isOutput=False)
    sel_e = nc.declare_dram_parameter("sel", [128, 256], bf16, isOutput=False)
    yT_e = nc.declare_dram_parameter("yT", [DIM, T], bf16, isOutput=True)

    with tile.TileContext(nc) as tc:
        with (
            tc.tile_pool(name="wt", bufs=1) as wt,
            tc.tile_pool(name="cst", bufs=1) as cst,
            tc.tile_pool(name="act", bufs=1) as act,
            tc.tile_pool(name="psA", bufs=8, space="PSUM") as psA,
        ):
            # ---- constants ----
            bqk = cst.tile([128, 16], f32)
            boc = cst.tile([128, NC], f32)
            b1c = cst.tile([128, 32], f32)
            b2c = cst.tile([128, NC], f32)
            sbo = cst.tile([1, 1], f32)
            sel = cst.tile([128, 256], bf16)
            wocs = cst.tile([128, 2 * NP, 1], f8)
            for dst, srcp in ((bqk, bqk_e), (boc, boc_e), (b1c, b1c_e),
                              (b2c, b2c_e), (sbo, sbo_e), (sel, sel_e),
                              (wocs, wocs_e)):
                nc.sync.dma_start(out=dst, in_=srcp[:])
            ones_q = cst.tile([128, 1], bf16)    # LN sum/sumsq lhsT
            ones_b = cst.tile([1, 128], bf16)    # K=1 broadcast lhsT
            eps_t = cst.tile([1, 1], f32)
            nl16 = cst.tile([128, 1], f32)
            nc.vector.memset(ones_q, 1.0)
            nc.vector.memset(ones_b, 1.0)
            nc.vector.memset(eps_t, EPS)
            nc.vector.memset(nl16, NEG_LN16)

            # per-iteration state handed to the delayed MLP section
            state = {}

            def emit_x_load(it):
                t0 = it * TT
                xt = [act.tile([128, TT], bf16, name=f"xt{c}", tag=f"xt{c}",
                               bufs=xin_bufs) for c in range(NC)]
                for c in range(NC):
                    nc.sync.dma_start(out=xt[c], in_=xT_e[c * 128:(c + 1) * 128,
                                                          t0:t0 + TT])
                return xt

            def emit_ln1_stats(xt):
                ps_s = psA.tile([1, TT], f32, name="ps_s1", tag="psA")
                ps_q = psA.tile([1, TT], f32, name="ps_q1", tag="psA")
                sqs = []
                for c in range(NC):
                    sq = act.tile([128, TT], bf16, name="sq1", tag="sq", bufs=2)
                    nc.vector.tensor_mul(sq, xt[c], xt[c])
                    sqs.append(sq)
                    nc.tensor.matmul(ps_s, lhsT=ones_q, rhs=xt[c],
                                     start=(c == 0), stop=(c == NC - 1))
                for c in range(NC):
                    nc.tensor.matmul(ps_q, lhsT=ones_q, rhs=sqs[c],
                                     start=(c == 0), stop=(c == NC - 1))
                m1b = act.tile([1, TT], bf16, name="m1b", tag="m1b")
                nc.scalar.activation(m1b, ps_s, AF.Copy, scale=1.0 / DIM)
                exq = act.tile([1, TT], bf16, name="exq1", tag="exq", bufs=2)
                nc.scalar.activation(exq, ps_q, AF.Copy, scale=1.0 / DIM)
                msq = act.tile([1, TT], bf16, name="msq1", tag="msq")
                nc.vector.tensor_mul(msq, m1b, m1b)
                nc.vector.tensor_sub(exq, exq, msq)
                nc.scalar.activation(exq, exq, AF.Ln, bias=eps_t)
                rs1 = act.tile([1, TT], bf16, name="rs1", tag="rs1")
                nc.scalar.activation(rs1, exq, AF.Exp, scale=-0.5)
                return m1b, rs1

            def emit_ln_bcast(mrow, rrow, nm):
                """Broadcast the per-token mean/rstd rows to all 128
                partitions as bf16 SBUF tiles (ones-matmul + Scalar copy),
                so the DVE apply ops run in the 2x all-16-bit mode."""
                mbc = act.tile([128, TT], bf16, name=f"mbc{nm}", tag="scb", bufs=4)
                rbc = act.tile([128, TT], bf16, name=f"rbc{nm}", tag="scb", bufs=4)
                for row, bc in ((mrow, mbc), (rrow, rbc)):
                    ps = psA.tile([128, TT], f32, name=f"ps_bc{nm}", tag="psA")
                    nc.tensor.matmul(ps, lhsT=ones_b, rhs=row, start=True, stop=True)
                    nc.scalar.activation(bc, ps, AF.Copy)
                return mbc, rbc

            def emit_ln1_apply(xt, m1b, rs1):
                mbc, rbc = emit_ln_bcast(m1b, rs1, "1")
                hp = act.tile([128, NC, TT], f8, name="hp", tag="p8")
                for c in range(NC):
                    cen = act.tile([128, TT], bf16, name="cen1", tag="cen", bufs=2)
                    nc.vector.tensor_sub(cen, xt[c], mbc)
                    nc.vector.tensor_mul(hp[:, c, :], cen, rbc)
                return hp

            def emit_mlp_ln(it):
                """LN2-apply for tile `it` (stats/r from emit_attn(it))."""
                st = state
                r_sb, m2row, r2row = st["r"], st["m2row"], st["r2row"]
                mbc, rbc = emit_ln_bcast(m2row, r2row, "2")
                h2b = []
                for c in range(NC):
                    cen = act.tile([128, TT], bf16, name="cen", tag="cen", bufs=2)
                    nc.vector.tensor_sub(cen, r_sb[c], mbc)
                    h2 = act.tile([128, TT], bf16, name=f"h2_{c}", tag=f"oh{c}")
                    nc.vector.tensor_mul(h2, cen, rbc)
                    h2b.append(h2)
                st["h2b"] = h2b

            def emit_w1(it):
                """W1 + gelu for tile `it`; W2 is woven into the next
                tile's attention groups (PE work under the exp shadow)."""
                st = state
                h2b = st["h2b"]
                g_sb = []
                for j in range(16):
                    g_sb.append(act.tile([128, 2, TT], f8, name=f"g{j}",
                                         tag=f"qkg{j}"))
                for hj in range(32):
                    ps = psA.tile([128, TT], f32, name="ps_w1", tag="psA")
                    for c in range(NC):
                        nc.tensor.matmul(ps, lhsT=w1_sb[c][:, hj * 128:(hj + 1) * 128],
                                         rhs=h2b[c], start=(c == 0), stop=(c == NC - 1))
                    nc.scalar.activation(g_sb[hj // 2][:, hj % 2, :], ps, AF.Gelu,
                                         bias=b1c[:, hj:hj + 1])
                st["g"] = g_sb
                st["t0_prev"] = it * TT
                st["r_prev"] = st["r"]

            def emit_w2_co(co):
                """One W2 output chunk of the previous tile."""
                st = state
                g_sb, r_sb, t0 = st["g"], st["r_prev"], st["t0_prev"]
                ps = psA.tile([128, TT], f32, name="ps_w2", tag="psA")
                for j in range(16):
                    nc.tensor.matmul(ps, lhsT=w2_sb[j][:, :, co * 128:(co + 1) * 128],
                                     rhs=g_sb[j], perf_mode=DR,
                                     start=(j == 0), stop=(j == 15))
                mo = act.tile([128, TT], bf16, name="mo", tag="tmp", bufs=2)
                nc.scalar.activation(mo, ps, AF.Identity, scale=1.0 / SW,
                                     bias=b2c[:, co:co + 1])
                nc.gpsimd.tensor_add(r_sb[co], r_sb[co], mo)
                nc.sync.dma_start(out=yT_e[co * 128:(co + 1) * 128, t0:t0 + TT],
                                  in_=r_sb[co])

            def emit_attn(it, xt, hp):
                """QKV + attention + Wo + LN2 stats for tile it."""
                # ---- QKV ----
                q_sb = [act.tile([128, TT], f8, name=f"q{c}", tag=f"q8_{c}")
                        for c in range(NC)]
                k_sb = [act.tile([128, TT], f8, name=f"k{c}", tag=f"k8_{c}")
                        for c in range(NC)]
                for co in range(NC):
                    ps = psA.tile([128, TT], f32, name="ps_q", tag="psA")
                    for j in range(NP):
                        nc.tensor.matmul(ps, lhsT=wq_sb[j][:, :, co * 128:(co + 1) * 128],
                                         rhs=hp[:, 2 * j:2 * j + 2, :], perf_mode=DR,
                                         start=(j == 0), stop=(j == NP - 1))
                    nc.vector.tensor_scalar(q_sb[co], ps, 8.0 / SQ,
                                            bqk[:, co:co + 1], ALU.mult, ALU.add)
                    ps = psA.tile([128, TT], f32, name="ps_k", tag="psA")
                    for j in range(NP):
                        nc.tensor.matmul(ps, lhsT=wk_sb[j][:, :, co * 128:(co + 1) * 128],
                                         rhs=hp[:, 2 * j:2 * j + 2, :], perf_mode=DR,
                                         start=(j == 0), stop=(j == NP - 1))
                    nc.vector.tensor_scalar(k_sb[co], ps, 8.0 / SW,
                                            bqk[:, 8 + co:8 + co + 1], ALU.mult, ALU.add)
                v_sb = [act.tile([128, HEADS, 2, 80], f8, name=f"v{w}", tag=f"v{w}")
                        for w in range(2)]
                for tc_ in range(4):
                    w, i = tc_ // 2, tc_ % 2
                    for nh in range(2):
                        ps = psA.tile([128, TT], f32, name="ps_v", tag="psA")
                        for j in range(NP):
                            nc.tensor.matmul(
                                ps, lhsT=hp[:, 2 * j:2 * j + 2, tc_ * 128:(tc_ + 1) * 128],
                                rhs=wv_sb[j][:, :, nh * 512:(nh + 1) * 512],
                                perf_mode=DR, start=(j == 0), stop=(j == NP - 1))
                        nc.vector.tensor_scalar_mul(
                            v_sb[w][:, nh * 8:(nh + 1) * 8, i, 0:64],
                            ps.rearrange("p (h d) -> p h d", d=64), 1.0 / SW)
                for w in range(2):
                    nc.vector.memset(v_sb[w][:, :, :, 64:65], 1.0)

                # ---- attention: 4-head groups, o lags one group ----
                sc = [act.tile([128, TT], bf16, name=f"sc{g}", tag="scb", bufs=4)
                      for g in range(4)]
                for g in range(4):
                    nc.vector.memset(sc[g], 1.0)
                oT = [act.tile([128, TT], bf16, name=f"oT{c}", tag=f"oh{c}")
                      for c in range(NC)]

                def emit_o(w, h0, e_g):
                    ws = w * WS2
                    for k2 in range(2):
                        hpair = (h0 + 2 * k2, h0 + 2 * k2 + 1)
                        ps_o = psA.tile([65, TT], f32, name="ps_o", tag="psA")
                        for j, h in enumerate(hpair):
                            nc.tensor.matmul(
                                ps_o[:, j * WS2:(j + 1) * WS2],
                                lhsT=v_sb[w][:, h, :, 0:65],
                                rhs=e_g[h].rearrange("p (two n) -> p two n", two=2),
                                perf_mode=DR, start=(j == 0), stop=(j == 1))
                        for j, h in enumerate(hpair):
                            ch, hh = h // 2, 64 * (h % 2)
                            nc.vector.tensor_copy(
                                sc[h // 4][32 * (h % 4):32 * (h % 4) + 1, ws:ws + WS2],
                                ps_o[64:65, j * WS2:(j + 1) * WS2])
                            nc.any.tensor_copy(oT[ch][hh:hh + 64, ws:ws + WS2],
                                               ps_o[0:64, j * WS2:(j + 1) * WS2])

                pend = None
                gi = 0
                for w in range(2):
                    ws = w * WS2
                    for h0 in range(0, HEADS, 4):
                        e_g = {}
                        ps_s_g = {}
                        for h in range(h0, h0 + 4):
                            ch, hh = h // 2, 64 * (h % 2)
                            ps_s = psA.tile([128, TT], f32, name="ps_sT", tag="psA")
                            nc.tensor.matmul(ps_s[:, 0:WS2],
                                             lhsT=k_sb[ch][hh:hh + 64, ws:ws + 128],
                                             rhs=q_sb[ch][hh:hh + 64, ws:ws + WS2],
                                             start=True, stop=False)
                            nc.tensor.matmul(ps_s[:, WS2:TT],
                                             lhsT=k_sb[ch][hh:hh + 64, ws + 128:ws + WS2],
                                             rhs=q_sb[ch][hh:hh + 64, ws:ws + WS2],
                                             start=False, stop=True)
                            ps_s_g[h] = ps_s
                        if pend is not None:
                            emit_o(*pend)
                        for h in range(h0, h0 + 4):
                            e_sb = act.tile([128, TT], f8, name="e_sb", tag="e", bufs=5)
                            nc.scalar.activation(e_sb, ps_s_g[h], AF.Exp, scale=1.0 / 64.0, bias=nl16)
                            e_g[h] = e_sb
                        if "g" in state:
                            emit_w2_co(gi)
                        gi += 1
                        pend = (w, h0, e_g)
                emit_o(*pend)
                state.pop("g", None)

                # ---- 1/s + normalize -> fp8 pairs ----
                with nc.allow_low_precision(reason="1/s as bf16 matmul operand"):
                    for g in range(4):
                        nc.scalar.activation(sc[g], sc[g], AF.Ln)
                        nc.scalar.activation(sc[g], sc[g], AF.Exp, scale=-1.0)
                oTp = act.tile([128, NC, TT], f8, name="oTp", tag="p8")
                for j in range(NC):
                    ps_b = psA.tile([128, TT], f32, name="ps_rsb", tag="psA")
                    nc.tensor.matmul(ps_b, lhsT=sel[:, 128 * (j % 2):128 * (j % 2) + 128],
                                     rhs=sc[j // 2], start=True, stop=True)
                    nc.vector.tensor_mul(oTp[:, j, :], oT[j], ps_b)

                # ---- LN2 mean (colsum over oTp; reuses LN1's sum) ----
                ps_s2 = psA.tile([1, TT], f32, name="ps_s2", tag="psA")
                for c in range(NC):
                    nc.tensor.matmul(ps_s2, lhsT=wocs[:, c, :], rhs=oTp[:, c, :],
                                     start=(c == 0), stop=(c == NC - 1))
                # m2 = m1 + sum_o/(SCS*DIM) + sum_bo/DIM
                m2f = act.tile([1, TT], bf16, name="m2f", tag="exq", bufs=2)
                nc.vector.tensor_scalar(m2f, ps_s2, 1.0 / (SCS * DIM),
                                        sbo, ALU.mult, ALU.add)
                m2row = act.tile([1, TT], bf16, name="m2row", tag="m2row", bufs=2)
                nc.vector.tensor_add(m2row, m2f, m1b)

                # ---- Wo + residual (LN2 sumsq pipelined per chunk) ----
                r_sb = [act.tile([128, TT], bf16, name=f"r{c}", tag=f"r{c}")
                        for c in range(NC)]
                ps_q2 = psA.tile([1, TT], f32, name="ps_q2", tag="psA")
                for co in range(NC):
                    ps = psA.tile([128, TT], f32, name="ps_wo", tag="psA")
                    for j in range(NP):
                        nc.tensor.matmul(ps, lhsT=wo_sb[j][:, :, co * 128:(co + 1) * 128],
                                         rhs=oTp[:, 2 * j:2 * j + 2, :], perf_mode=DR,
                                         start=(j == 0), stop=(j == NP - 1))
                    wos = act.tile([128, TT], bf16, name="wos", tag="tmp", bufs=2)
                    nc.scalar.activation(wos, ps, AF.Identity, scale=1.0 / SW,
                                         bias=boc[:, co:co + 1])
                    nc.vector.tensor_add(r_sb[co], xt[co], wos)
                    sq = act.tile([128, TT], bf16, name="sq2", tag="sq", bufs=2)
                    nc.vector.tensor_mul(sq, r_sb[co], r_sb[co])
                    nc.tensor.matmul(ps_q2, lhsT=ones_q, rhs=sq,
                                     start=(co == 0), stop=(co == NC - 1))
                exq = act.tile([1, TT], bf16, name="exq2", tag="exq", bufs=2)
                nc.scalar.activation(exq, ps_q2, AF.Copy, scale=1.0 / DIM)
                msq = act.tile([1, TT], bf16, name="msq2", tag="msq")
                nc.vector.tensor_mul(msq, m2row, m2row)
                nc.vector.tensor_sub(exq, exq, msq)
                nc.scalar.activation(exq, exq, AF.Ln, bias=eps_t)
                r2row = act.tile([1, TT], bf16, name="r2row", tag="r2row", bufs=2)
                nc.scalar.activation(r2row, exq, AF.Exp, scale=-0.5)
                state.update(r=r_sb, m2row=m2row, r2row=r2row)

            # ================= fused, software-pipelined pass =============
            xt_cur = emit_x_load(0)

            # ---- resident weights (after x(0) so tile 0 starts early) ----
            wq_sb, wk_sb, wv_sb, wo_sb = [], [], [], []
            for lst, src, nm in ((wq_sb, wqp_e, "wq"), (wk_sb, wkp_e, "wk"),
                                 (wv_sb, wvp_e, "wv"), (wo_sb, wop_e, "wo")):
                for j in range(NP):
                    t_ = wt.tile([128, 2, DIM], f8, name=f"{nm}{j}")
                    nc.sync.dma_start(out=t_, in_=src[:, 2 * j:2 * j + 2, :])
                    lst.append(t_)
            w1_sb = []
            for c in range(NC):
                t_ = wt.tile([128, HID], bf16, name=f"w1_{c}")
                nc.sync.dma_start(out=t_, in_=w1_e[c * 128:(c + 1) * 128, :])
                w1_sb.append(t_)
            w2_sb = []
            for j in range(16):
                t_ = wt.tile([128, 2, DIM], f8, name=f"w2_{j}")
                nc.sync.dma_start(out=t_, in_=w2p_e[:, 2 * j:2 * j + 2, :])
                w2_sb.append(t_)

            for it in range(NT):
                xt_next = emit_x_load(it + 1) if it + 1 < NT else None
                m1b, rs1 = emit_ln1_stats(xt_cur)
                if it > 0:
                    emit_mlp_ln(it - 1)
                hp = emit_ln1_apply(xt_cur, m1b, rs1)
                if it > 0:
                    emit_w1(it - 1)
                emit_attn(it, xt_cur, hp)
                xt_cur = xt_next
            emit_mlp_ln(NT - 1)
            emit_w1(NT - 1)
            for co in range(NC):
                emit_w2_co(co)

    _split_multi_waits(nc)
    return nc


# ---------------------------------------------------------------------------
# Host side
# ---------------------------------------------------------------------------
_CACHE = {}


def _bf(a):
    return np.ascontiguousarray(a).astype(ml_dtypes.bfloat16)


def _q8(w, s):
    """Quantize to TRN fp8e4 (max 240) with scale s, packed as 128-row pairs:
    out[r, 2j+i, c] = fp8(s * w[256j + 128i + r, c])."""
    q = np.clip(w * s, -240.0, 240.0).astype(ml_dtypes.float8_e4m3)
    K = w.shape[0]
    return np.ascontiguousarray(
        q.reshape(K // 128, 128, -1).transpose(1, 0, 2))


def prep_consts(g1, beta1, Wq, bq, Wk, bk, Wv, bv, Wo, bo, g2, beta2,
                W1, b1m, W2, b2m):
    Wq_e = (g1[:, None] * Wq) * SCALE
    bq_e = (beta1 @ Wq + bq) * SCALE
    Wk_e = g1[:, None] * Wk
    bk_e = beta1 @ Wk + bk
    Wv_e = g1[:, None] * Wv
    bv_e = beta1 @ Wv + bv
    bo_e = bv_e @ Wo + bo
    W1_e = g2[:, None] * W1
    b1_e = beta2 @ W1 + b1m
    bqk = np.concatenate([bq_e.reshape(8, 128).T, bk_e.reshape(8, 128).T], axis=1) * 8.0
    sel = np.zeros((128, 256), np.float32)
    sel[0, 0:64] = 1.0
    sel[32, 64:128] = 1.0
    sel[64, 128 + 0:128 + 64] = 1.0
    sel[96, 128 + 64:128 + 128] = 1.0
    wop = _q8(Wo, SW)
    # colsum of the QUANTIZED Wo (so LN2's mean matches the computed r)
    wo_deq = wop.astype(np.float32).transpose(1, 0, 2).reshape(DIM, DIM) / SW
    wocs = np.clip(wo_deq.sum(axis=1) * SCS, -240, 240).astype(ml_dtypes.float8_e4m3)
    wocs = np.ascontiguousarray(
        wocs.reshape(NP * 2, 128).T.reshape(128, NP * 2, 1))
    return {
        "wqp": _q8(Wq_e, SQ), "wkp": _q8(Wk_e, SW), "wvp": _q8(Wv_e, SW),
        "wop": wop, "wocs": wocs,
        "w1": _bf(W1_e), "w2p": _q8(W2, SW),
        "bqk": np.ascontiguousarray(bqk.astype(np.float32)),
        "boc": np.ascontiguousarray(bo_e.reshape(NC, 128).T.astype(np.float32)),
        "b1c": np.ascontiguousarray(b1_e.reshape(32, 128).T.astype(np.float32)),
        "b2c": np.ascontiguousarray(b2m.reshape(NC, 128).T.astype(np.float32)),
        "sbo": np.array([[bo_e.sum() / DIM]], np.float32),
        "sel": _bf(sel),
    }


def window_order(x_b):
    # [4096, C] row-major spatial -> window-contiguous [4096, C]
    C = x_b.shape[-1]
    t = x_b.reshape(4, 16, 4, 16, C).transpose(0, 2, 1, 3, 4)
    return t.reshape(4096, C)


def window_unorder(y_b):
    C = y_b.shape[-1]
    t = y_b.reshape(4, 4, 16, 16, C).transpose(0, 2, 1, 3, 4)
    return t.reshape(4096, C)


def kernel(x, g1, beta1, Wq, bq, Wk, bk, Wv, bv, Wo, bo, g2, beta2,
           W1, b1m, W2, b2m, window_size, spatial_h, spatial_w):
    x = np.asarray(x, np.float32)
    args = [np.asarray(a, np.float32) for a in
            (g1, beta1, Wq, bq, Wk, bk, Wv, bv, Wo, bo, g2, beta2, W1, b1m, W2, b2m)]
    consts = prep_consts(*args)

    if "nc" not in _CACHE:
        _CACHE["nc"] = build_nc(NT=8)
    nc = _CACHE["nc"]

    B = x.shape[0]
    in_maps = []
    for c in range(B):
        xw = window_order(x[c])                       # [4096, C]
        m = {"xT": np.ascontiguousarray(xw.T).astype(ml_dtypes.bfloat16)}
        m.update(consts)
        in_maps.append(m)
    res = run_bass_kernel_spmd(nc, in_maps, core_ids=list(range(B)))
    out = np.empty_like(x)
    for c in range(B):
        yT = res.results[c]["yT"].astype(np.float32)  # [C, 4096]
        out[c] = window_unorder(np.ascontiguousarray(yT.T))
    return out


# revision 38
# speedup vs baseline: 1.0609x; 1.0083x over previous
"""Trainium2 Bass kernel for nn_BlockDrop (Swin-style transformer block).

Reference math (per batch image):
  h = LN1(x); 16x16 windows of 256 tokens; 16-head attention (d=64) with
  separate Q/K/V/O linears; x += attn; h2 = LN2(x); x += W2@gelu(W1@h2).

Sharding: pure data parallel — batch image b -> core b (16 windows each,
no cross-core communication). Host performs window reordering,
transposition (feature-major) and weight folding.

v2: fp8e4 DoubleRow matmuls (2x PE throughput) for QKV, Wo, attn@V and
W2; scores and W1 stay bf16 for the error budget. fp8 weights fit all
six weight sets in SBUF simultaneously, so the kernel runs ONE fused
pass per 512-token tile with no DRAM round-trips for intermediates.
Software pipelining: the MLP of tile i-1 is emitted between tile i's
LN1 stats and its attention, hiding LN post-processing latency.
LayerNorm stats via ones-matmuls; LN2 mean reuses LN1's sum plus a
rank-1 colsum matmul over the attention output (no f32 sum matmul).
Softmax: scores^T, exp scaled by 1/16 into fp8 (denominator via a ones
column in V, cancels exactly); 1/s broadcast by selector matmuls.
"""
import numpy as np
import ml_dtypes

import concourse.bass as bass
import concourse.mybir as mybir
import concourse.tile as tile
from concourse.bass_utils import run_bass_kernel_spmd

f32 = mybir.dt.float32
bf16 = mybir.dt.bfloat16
f8 = mybir.dt.float8e4
AF = mybir.ActivationFunctionType
ALU = mybir.AluOpType
DR = mybir.MatmulPerfMode.DoubleRow

DIM = 1024
HEADS = 16
HDIM = 64
HID = 4096
SCALE = HDIM ** -0.5
EPS = 1e-5
T = 4096          # tokens per core
TT = 512          # tokens per T-tile (2 windows)
NC = 8            # 128-ch chunks of DIM
NP = 4            # 256-ch pairs of DIM
WS2 = 256         # tokens per window

SQ = 512.0        # fp8 scale for Wq (includes 1/8 attention scale)
SW = 64.0         # fp8 scale for Wk/Wv/Wo/W2
SCS = 16.0        # fp8 scale for wo colsum
NEG_LN16 = -2.7725887  # exp bias so e' = exp(s)/16 stays in fp8 range


def _split_multi_waits(nc):
    """This walrus rejects >1 sync-wait per instruction. Move extra waits
    onto same-engine NoOps inserted just before (engine queues are FIFO,
    so blocking the queue on each sem in turn is equivalent)."""
    n_split = 0
    for fn in nc.m.functions:
        for blk in fn.blocks:
            insts = blk.instructions
            new = []
            for inst in insts:
                si = inst.sync_info
                waits = list(si.on_wait) if si is not None else []
                if len(waits) > 1:
                    for w in waits[:-1]:
                        n_split += 1
                        new.append(mybir.InstNoOp(
                            name=f"{inst.name}-ws{n_split}",
                            engine=inst.engine, ins=[], outs=[],
                            sync_info=mybir.SyncInfo(on_wait=[w], on_update=[]),
                        ))
                    inst.sync_info = mybir.SyncInfo(
                        on_wait=[waits[-1]], on_update=list(si.on_update))
                new.append(inst)
            if len(new) != len(insts):
                blk.instructions[:] = new
    return n_split


def build_nc(NT=8, use_f32r=False, xin_bufs=1):
    nc = bass.Bass()

    xT_e = nc.declare_dram_parameter("xT", [DIM, T], bf16, isOutput=False)
    wqp_e = nc.declare_dram_parameter("wqp", [128, 2 * NP, DIM], f8, isOutput=False)
    wkp_e = nc.declare_dram_parameter("wkp", [128, 2 * NP, DIM], f8, isOutput=False)
    wvp_e = nc.declare_dram_parameter("wvp", [128, 2 * NP, DIM], f8, isOutput=False)
    wop_e = nc.declare_dram_parameter("wop", [128, 2 * NP, DIM], f8, isOutput=False)
    w1_e = nc.declare_dram_parameter("w1", [DIM, HID], bf16, isOutput=False)
    w2p_e = nc.declare_dram_parameter("w2p", [128, 32, DIM], f8, isOutput=False)
    wocs_e = nc.declare_dram_parameter("wocs", [128, 2 * NP, 1], f8, isOutput=False)
    bqk_e = nc.declare_dram_parameter("bqk", [128, 16], f32, isOutput=False)
    boc_e = nc.declare_dram_parameter("boc", [128, NC], f32, isOutput=False)
    b1c_e = nc.declare_dram_parameter("b1c", [128, 32], f32, isOutput=False)
    b2c_e = nc.declare_dram_parameter("b2c", [128, NC], f32, isOutput=False)
    sbo_e = nc.declare_dram_parameter("sbo", [1, 1], f32, isOutput=False)
    sel_e = nc.declare_dram_parameter("sel", [128, 256], bf16, isOutput=False)
    yT_e = nc.declare_dram_parameter("yT", [DIM, T], bf16, isOutput=True)

    with tile.TileContext(nc) as tc:
        with (
            tc.tile_pool(name="wt", bufs=1) as wt,
            tc.tile_pool(name="cst", bufs=1) as cst,
            tc.tile_pool(name="act", bufs=1) as act,
            tc.tile_pool(name="psA", bufs=8, space="PSUM") as psA,
        ):
            # ---- constants ----
            bqk = cst.tile([128, 16], f32)
            boc = cst.tile([128, NC], f32)
            b1c = cst.tile([128, 32], f32)
            b2c = cst.tile([128, NC], f32)
            sbo = cst.tile([1, 1], f32)
            sel = cst.tile([128, 256], bf16)
            wocs = cst.tile([128, 2 * NP, 1], f8)
            for dst, srcp in ((bqk, bqk_e), (boc, boc_e), (b1c, b1c_e),
                              (b2c, b2c_e), (sbo, sbo_e), (sel, sel_e),
                              (wocs, wocs_e)):
                nc.sync.dma_start(out=dst, in_=srcp[:])
            ones_q = cst.tile([128, 1], bf16)    # LN sum/sumsq lhsT
            ones_b = cst.tile([1, 128], bf16)    # K=1 broadcast lhsT
            eps_t = cst.tile([1, 1], f32)
            nl16 = cst.tile([128, 1], f32)
            nc.vector.memset(ones_q, 1.0)
            nc.vector.memset(ones_b, 1.0)
            nc.vector.memset(eps_t, EPS)
            nc.vector.memset(nl16, NEG_LN16)

            # per-iteration state handed to the delayed MLP section
            state = {}

            def emit_x_load(it):
                t0 = it * TT
                xt = [act.tile([128, TT], bf16, name=f"xt{c}", tag=f"xt{c}",
                               bufs=xin_bufs) for c in range(NC)]
                for c in range(NC):
                    nc.sync.dma_start(out=xt[c], in_=xT_e[c * 128:(c + 1) * 128,
                                                          t0:t0 + TT])
                return xt

            def emit_ln1_stats(xt):
                ps_s = psA.tile([1, TT], f32, name="ps_s1", tag="psA")
                ps_q = psA.tile([1, TT], f32, name="ps_q1", tag="psA")
                sqs = []
                for c in range(NC):
                    sq = act.tile([128, TT], bf16, name="sq1", tag="sq", bufs=2)
                    nc.vector.tensor_mul(sq, xt[c], xt[c])
                    sqs.append(sq)
                    nc.tensor.matmul(ps_s, lhsT=ones_q, rhs=xt[c],
                                     start=(c == 0), stop=(c == NC - 1))
                for c in range(NC):
                    nc.tensor.matmul(ps_q, lhsT=ones_q, rhs=sqs[c],
                                     start=(c == 0), stop=(c == NC - 1))
                m1b = act.tile([1, TT], bf16, name="m1b", tag="m1b")
                nc.scalar.activation(m1b, ps_s, AF.Copy, scale=1.0 / DIM)
                exq = act.tile([1, TT], bf16, name="exq1", tag="exq", bufs=2)
                nc.scalar.activation(exq, ps_q, AF.Copy, scale=1.0 / DIM)
                msq = act.tile([1, TT], bf16, name="msq1", tag="msq")
                nc.vector.tensor_mul(msq, m1b, m1b)
                nc.vector.tensor_sub(exq, exq, msq)
                nc.scalar.activation(exq, exq, AF.Ln, bias=eps_t)
                rs1 = act.tile([1, TT], bf16, name="rs1", tag="rs1")
                nc.scalar.activation(rs1, exq, AF.Exp, scale=-0.5)
                return m1b, rs1

            def emit_ln_bcast(mrow, rrow, nm):
                """Broadcast the per-token mean/rstd rows to all 128
                partitions as bf16 SBUF tiles (ones-matmul + Scalar copy),
                so the DVE apply ops run in the 2x all-16-bit mode."""
                mbc = act.tile([128, TT], bf16, name=f"mbc{nm}", tag="scb", bufs=4)
                rbc = act.tile([128, TT], bf16, name=f"rbc{nm}", tag="scb", bufs=4)
                for row, bc in ((mrow, mbc), (rrow, rbc)):
                    ps = psA.tile([128, TT], f32, name=f"ps_bc{nm}", tag="psA")
                    nc.tensor.matmul(ps, lhsT=ones_b, rhs=row, start=True, stop=True)
                    nc.scalar.activation(bc, ps, AF.Copy)
                return mbc, rbc

            def emit_ln1_apply(xt, m1b, rs1):
                mbc, rbc = emit_ln_bcast(m1b, rs1, "1")
                hp = act.tile([128, NC, TT], f8, name="hp", tag="p8")
                for c in range(NC):
                    cen = act.tile([128, TT], bf16, name="cen1", tag="cen", bufs=2)
                    nc.vector.tensor_sub(cen, xt[c], mbc)
                    nc.vector.tensor_mul(hp[:, c, :], cen, rbc)
                return hp

            def emit_mlp_ln(it):
                """LN2-apply for tile `it` (stats/r from emit_attn(it))."""
                st = state
                r_sb, m2row, r2row = st["r"], st["m2row"], st["r2row"]
                mbc, rbc = emit_ln_bcast(m2row, r2row, "2")
                h2b = []
                for c in range(NC):
                    cen = act.tile([128, TT], bf16, name="cen", tag="cen", bufs=2)
                    nc.vector.tensor_sub(cen, r_sb[c], mbc)
                    h2 = act.tile([128, TT], bf16, name=f"h2_{c}", tag=f"oh{c}")
                    nc.vector.tensor_mul(h2, cen, rbc)
                    h2b.append(h2)
                st["h2b"] = h2b

            def emit_w1(it):
                """W1 + gelu for tile `it`; W2 is woven into the next
                tile's attention groups (PE work under the exp shadow)."""
                st = state
                h2b = st["h2b"]
                g_sb = []
                for j in range(16):
                    g_sb.append(act.tile([128, 2, TT], f8, name=f"g{j}",
                                         tag=f"qkg{j}"))
                for hj in range(32):
                    ps = psA.tile([128, TT], f32, name="ps_w1", tag="psA")
                    for c in range(NC):
                        nc.tensor.matmul(ps, lhsT=w1_sb[c][:, hj * 128:(hj + 1) * 128],
                                         rhs=h2b[c], start=(c == 0), stop=(c == NC - 1))
                    nc.scalar.activation(g_sb[hj // 2][:, hj % 2, :], ps, AF.Gelu,
                                         bias=b1c[:, hj:hj + 1])
                st["g"] = g_sb
                st["t0_prev"] = it * TT
                st["r_prev"] = st["r"]

            def emit_w2_co(co):
                """One W2 output chunk of the previous tile."""
                st = state
                g_sb, r_sb, t0 = st["g"], st["r_prev"], st["t0_prev"]
                ps = psA.tile([128, TT], f32, name="ps_w2", tag="psA")
                for j in range(16):
                    nc.tensor.matmul(ps, lhsT=w2_sb[j][:, :, co * 128:(co + 1) * 128],
                                     rhs=g_sb[j], perf_mode=DR,
                                     start=(j == 0), stop=(j == 15))
                mo = act.tile([128, TT], bf16, name="mo", tag="tmp", bufs=2)
                nc.scalar.activation(mo, ps, AF.Identity, scale=1.0 / SW,
                                     bias=b2c[:, co:co + 1])
                nc.gpsimd.tensor_add(r_sb[co], r_sb[co], mo)
                nc.sync.dma_start(out=yT_e[co * 128:(co + 1) * 128, t0:t0 + TT],
                                  in_=r_sb[co])

            def emit_attn(it, xt, hp):
                """QKV + attention + Wo + LN2 stats for tile it."""
                # ---- QKV ----
                q_sb = [act.tile([128, TT], f8, name=f"q{c}", tag=f"q8_{c}")
                        for c in range(NC)]
                k_sb = [act.tile([128, TT], f8, name=f"k{c}", tag=f"k8_{c}")
                        for c in range(NC)]
                for co in range(NC):
                    ps = psA.tile([128, TT], f32, name="ps_q", tag="psA")
                    for j in range(NP):
                        nc.tensor.matmul(ps, lhsT=wq_sb[j][:, :, co * 128:(co + 1) * 128],
                                         rhs=hp[:, 2 * j:2 * j + 2, :], perf_mode=DR,
                                         start=(j == 0), stop=(j == NP - 1))
                    nc.vector.tensor_scalar(q_sb[co], ps, 8.0 / SQ,
                                            bqk[:, co:co + 1], ALU.mult, ALU.add)
                    ps = psA.tile([128, TT], f32, name="ps_k", tag="psA")
                    for j in range(NP):
                        nc.tensor.matmul(ps, lhsT=wk_sb[j][:, :, co * 128:(co + 1) * 128],
                                         rhs=hp[:, 2 * j:2 * j + 2, :], perf_mode=DR,
                                         start=(j == 0), stop=(j == NP - 1))
                    nc.vector.tensor_scalar(k_sb[co], ps, 8.0 / SW,
                                            bqk[:, 8 + co:8 + co + 1], ALU.mult, ALU.add)
                v_sb = [act.tile([128, HEADS, 2, 80], f8, name=f"v{w}", tag=f"v{w}")
                        for w in range(2)]
                for tc_ in range(4):
                    w, i = tc_ // 2, tc_ % 2
                    for nh in range(2):
                        ps = psA.tile([128, TT], f32, name="ps_v", tag="psA")
                        for j in range(NP):
                            nc.tensor.matmul(
                                ps, lhsT=hp[:, 2 * j:2 * j + 2, tc_ * 128:(tc_ + 1) * 128],
                                rhs=wv_sb[j][:, :, nh * 512:(nh + 1) * 512],
                                perf_mode=DR, start=(j == 0), stop=(j == NP - 1))
                        nc.vector.tensor_scalar_mul(
                            v_sb[w][:, nh * 8:(nh + 1) * 8, i, 0:64],
                            ps.rearrange("p (h d) -> p h d", d=64), 1.0 / SW)
                for w in range(2):
                    nc.vector.memset(v_sb[w][:, :, :, 64:65], 1.0)

                # ---- attention: 4-head groups, o lags one group ----
                sc = [act.tile([128, TT], bf16, name=f"sc{g}", tag="scb", bufs=4)
                      for g in range(4)]
                for g in range(4):
                    nc.vector.memset(sc[g], 1.0)
                oT = [act.tile([128, TT], bf16, name=f"oT{c}", tag=f"oh{c}")
                      for c in range(NC)]

                def emit_o(w, h0, e_g):
                    ws = w * WS2
                    g4 = h0 // 4
                    for k2 in range(2):
                        hpair = (h0 + 2 * k2, h0 + 2 * k2 + 1)
                        ps_o = psA.tile([65, TT], f32, name="ps_o", tag="psA")
                        for j, h in enumerate(hpair):
                            nc.tensor.matmul(
                                ps_o[:, j * WS2:(j + 1) * WS2],
                                lhsT=v_sb[w][:, h, :, 0:65],
                                rhs=e_g[h].rearrange("p (two n) -> p two n", two=2),
                                perf_mode=DR, start=(j == 0), stop=(j == 1))
                        for j, h in enumerate(hpair):
                            ch, hh = h // 2, 64 * (h % 2)
                            nc.vector.tensor_copy(
                                sc[h // 4][32 * (h % 4):32 * (h % 4) + 1, ws:ws + WS2],
                                ps_o[64:65, j * WS2:(j + 1) * WS2])
                            nc.any.tensor_copy(oT[ch][hh:hh + 64, ws:ws + WS2],
                                               ps_o[0:64, j * WS2:(j + 1) * WS2])
                    if w == 1:
                        with nc.allow_low_precision(reason="1/s as bf16 operand"):
                            nc.scalar.activation(sc[g4], sc[g4], AF.Ln)
                            nc.scalar.activation(sc[g4], sc[g4], AF.Exp, scale=-1.0)

                pend = None
                gi = 0
                for w in range(2):
                    ws = w * WS2
                    for h0 in range(0, HEADS, 4):
                        e_g = {}
                        ps_s_g = {}
                        for h in range(h0, h0 + 4):
                            ch, hh = h // 2, 64 * (h % 2)
                            ps_s = psA.tile([128, TT], f32, name="ps_sT", tag="psA")
                            nc.tensor.matmul(ps_s[:, 0:WS2],
                                             lhsT=k_sb[ch][hh:hh + 64, ws:ws + 128],
                                             rhs=q_sb[ch][hh:hh + 64, ws:ws + WS2],
                                             start=True, stop=False)
                            nc.tensor.matmul(ps_s[:, WS2:TT],
                                             lhsT=k_sb[ch][hh:hh + 64, ws + 128:ws + WS2],
                                             rhs=q_sb[ch][hh:hh + 64, ws:ws + WS2],
                                             start=False, stop=True)
                            ps_s_g[h] = ps_s
                        if pend is not None:
                            emit_o(*pend)
                        for h in range(h0, h0 + 4):
                            e_sb = act.tile([128, TT], f8, name="e_sb", tag="e", bufs=5)
                            nc.scalar.activation(e_sb, ps_s_g[h], AF.Exp, scale=1.0 / 64.0, bias=nl16)
                            e_g[h] = e_sb
                        if "g" in state:
                            emit_w2_co(gi)
                        gi += 1
                        pend = (w, h0, e_g)
                emit_o(*pend)
                state.pop("g", None)

                # ---- normalize -> fp8 pairs (1/s computed in emit_o) ----
                oTp = act.tile([128, NC, TT], f8, name="oTp", tag="p8")
                for j in range(NC):
                    ps_b = psA.tile([128, TT], f32, name="ps_rsb", tag="psA")
                    nc.tensor.matmul(ps_b, lhsT=sel[:, 128 * (j % 2):128 * (j % 2) + 128],
                                     rhs=sc[j // 2], start=True, stop=True)
                    nc.vector.tensor_mul(oTp[:, j, :], oT[j], ps_b)

                # ---- LN2 mean (colsum over oTp; reuses LN1's sum) ----
                ps_s2 = psA.tile([1, TT], f32, name="ps_s2", tag="psA")
                for c in range(NC):
                    nc.tensor.matmul(ps_s2, lhsT=wocs[:, c, :], rhs=oTp[:, c, :],
                                     start=(c == 0), stop=(c == NC - 1))
                # m2 = m1 + sum_o/(SCS*DIM) + sum_bo/DIM
                m2f = act.tile([1, TT], bf16, name="m2f", tag="exq", bufs=2)
                nc.vector.tensor_scalar(m2f, ps_s2, 1.0 / (SCS * DIM),
                                        sbo, ALU.mult, ALU.add)
                m2row = act.tile([1, TT], bf16, name="m2row", tag="m2row", bufs=2)
                nc.vector.tensor_add(m2row, m2f, m1b)

                # ---- Wo + residual (LN2 sumsq pipelined per chunk) ----
                r_sb = [act.tile([128, TT], bf16, name=f"r{c}", tag=f"r{c}")
                        for c in range(NC)]
                ps_q2 = psA.tile([1, TT], f32, name="ps_q2", tag="psA")
                for co in range(NC):
                    ps = psA.tile([128, TT], f32, name="ps_wo", tag="psA")
                    for j in range(NP):
                        nc.tensor.matmul(ps, lhsT=wo_sb[j][:, :, co * 128:(co + 1) * 128],
                                         rhs=oTp[:, 2 * j:2 * j + 2, :], perf_mode=DR,
                                         start=(j == 0), stop=(j == NP - 1))
                    wos = act.tile([128, TT], bf16, name="wos", tag="tmp", bufs=2)
                    nc.scalar.activation(wos, ps, AF.Identity, scale=1.0 / SW,
                                         bias=boc[:, co:co + 1])
                    nc.vector.tensor_add(r_sb[co], xt[co], wos)
                    sq = act.tile([128, TT], bf16, name="sq2", tag="sq", bufs=2)
                    nc.vector.tensor_mul(sq, r_sb[co], r_sb[co])
                    nc.tensor.matmul(ps_q2, lhsT=ones_q, rhs=sq,
                                     start=(co == 0), stop=(co == NC - 1))
                exq = act.tile([1, TT], bf16, name="exq2", tag="exq", bufs=2)
                nc.scalar.activation(exq, ps_q2, AF.Copy, scale=1.0 / DIM)
                msq = act.tile([1, TT], bf16, name="msq2", tag="msq")
                nc.vector.tensor_mul(msq, m2row, m2row)
                nc.vector.tensor_sub(exq, exq, msq)
                nc.scalar.activation(exq, exq, AF.Ln, bias=eps_t)
                r2row = act.tile([1, TT], bf16, name="r2row", tag="r2row", bufs=2)
                nc.scalar.activation(r2row, exq, AF.Exp, scale=-0.5)
                state.update(r=r_sb, m2row=m2row, r2row=r2row)

            # ================= fused, software-pipelined pass =============
            xt_cur = emit_x_load(0)

            # ---- resident weights (after x(0) so tile 0 starts early) ----
            wq_sb, wk_sb, wv_sb, wo_sb = [], [], [], []
            for lst, src, nm in ((wq_sb, wqp_e, "wq"), (wk_sb, wkp_e, "wk"),
                                 (wv_sb, wvp_e, "wv"), (wo_sb, wop_e, "wo")):
                for j in range(NP):
                    t_ = wt.tile([128, 2, DIM], f8, name=f"{nm}{j}")
                    nc.sync.dma_start(out=t_, in_=src[:, 2 * j:2 * j + 2, :])
                    lst.append(t_)
            w1_sb = []
            for c in range(NC):
                t_ = wt.tile([128, HID], bf16, name=f"w1_{c}")
                nc.sync.dma_start(out=t_, in_=w1_e[c * 128:(c + 1) * 128, :])
                w1_sb.append(t_)
            w2_sb = []
            for j in range(16):
                t_ = wt.tile([128, 2, DIM], f8, name=f"w2_{j}")
                nc.sync.dma_start(out=t_, in_=w2p_e[:, 2 * j:2 * j + 2, :])
                w2_sb.append(t_)

            for it in range(NT):
                xt_next = emit_x_load(it + 1) if it + 1 < NT else None
                m1b, rs1 = emit_ln1_stats(xt_cur)
                if it > 0:
                    emit_mlp_ln(it - 1)
                hp = emit_ln1_apply(xt_cur, m1b, rs1)
                if it > 0:
                    emit_w1(it - 1)
                emit_attn(it, xt_cur, hp)
                xt_cur = xt_next
            emit_mlp_ln(NT - 1)
            emit_w1(NT - 1)
            for co in range(NC):
                emit_w2_co(co)

    _split_multi_waits(nc)
    return nc


# ---------------------------------------------------------------------------
# Host side
# ---------------------------------------------------------------------------
_CACHE = {}


def _bf(a):
    return np.ascontiguousarray(a).astype(ml_dtypes.bfloat16)


def _q8(w, s):
    """Quantize to TRN fp8e4 (max 240) with scale s, packed as 128-row pairs:
    out[r, 2j+i, c] = fp8(s * w[256j + 128i + r, c])."""
    q = np.clip(w * s, -240.0, 240.0).astype(ml_dtypes.float8_e4m3)
    K = w.shape[0]
    return np.ascontiguousarray(
        q.reshape(K // 128, 128, -1).transpose(1, 0, 2))


def prep_consts(g1, beta1, Wq, bq, Wk, bk, Wv, bv, Wo, bo, g2, beta2,
                W1, b1m, W2, b2m):
    Wq_e = (g1[:, None] * Wq) * SCALE
    bq_e = (beta1 @ Wq + bq) * SCALE
    Wk_e = g1[:, None] * Wk
    bk_e = beta1 @ Wk + bk
    Wv_e = g1[:, None] * Wv
    bv_e = beta1 @ Wv + bv
    bo_e = bv_e @ Wo + bo
    W1_e = g2[:, None] * W1
    b1_e = beta2 @ W1 + b1m
    bqk = np.concatenate([bq_e.reshape(8, 128).T, bk_e.reshape(8, 128).T], axis=1) * 8.0
    sel = np.zeros((128, 256), np.float32)
    sel[0, 0:64] = 1.0
    sel[32, 64:128] = 1.0
    sel[64, 128 + 0:128 + 64] = 1.0
    sel[96, 128 + 64:128 + 128] = 1.0
    wop = _q8(Wo, SW)
    # colsum of the QUANTIZED Wo (so LN2's mean matches the computed r)
    wo_deq = wop.astype(np.float32).transpose(1, 0, 2).reshape(DIM, DIM) / SW
    wocs = np.clip(wo_deq.sum(axis=1) * SCS, -240, 240).astype(ml_dtypes.float8_e4m3)
    wocs = np.ascontiguousarray(
        wocs.reshape(NP * 2, 128).T.reshape(128, NP * 2, 1))
    return {
        "wqp": _q8(Wq_e, SQ), "wkp": _q8(Wk_e, SW), "wvp": _q8(Wv_e, SW),
        "wop": wop, "wocs": wocs,
        "w1": _bf(W1_e), "w2p": _q8(W2, SW),
        "bqk": np.ascontiguousarray(bqk.astype(np.float32)),
        "boc": np.ascontiguousarray(bo_e.reshape(NC, 128).T.astype(np.float32)),
        "b1c": np.ascontiguousarray(b1_e.reshape(32, 128).T.astype(np.float32)),
        "b2c": np.ascontiguousarray(b2m.reshape(NC, 128).T.astype(np.float32)),
        "sbo": np.array([[bo_e.sum() / DIM]], np.float32),
        "sel": _bf(sel),
    }


def window_order(x_b):
    # [4096, C] row-major spatial -> window-contiguous [4096, C]
    C = x_b.shape[-1]
    t = x_b.reshape(4, 16, 4, 16, C).transpose(0, 2, 1, 3, 4)
    return t.reshape(4096, C)


def window_unorder(y_b):
    C = y_b.shape[-1]
    t = y_b.reshape(4, 4, 16, 16, C).transpose(0, 2, 1, 3, 4)
    return t.reshape(4096, C)


def kernel(x, g1, beta1, Wq, bq, Wk, bk, Wv, bv, Wo, bo, g2, beta2,
           W1, b1m, W2, b2m, window_size, spatial_h, spatial_w):
    x = np.asarray(x, np.float32)
    args = [np.asarray(a, np.float32) for a in
            (g1, beta1, Wq, bq, Wk, bk, Wv, bv, Wo, bo, g2, beta2, W1, b1m, W2, b2m)]
    consts = prep_consts(*args)

    if "nc" not in _CACHE:
        _CACHE["nc"] = build_nc(NT=8)
    nc = _CACHE["nc"]

    B = x.shape[0]
    in_maps = []
    for c in range(B):
        xw = window_order(x[c])                       # [4096, C]
        m = {"xT": np.ascontiguousarray(xw.T).astype(ml_dtypes.bfloat16)}
        m.update(consts)
        in_maps.append(m)
    res = run_bass_kernel_spmd(nc, in_maps, core_ids=list(range(B)))
    out = np.empty_like(x)
    for c in range(B):
        yT = res.results[c]["yT"].astype(np.float32)  # [C, 4096]
        out[c] = window_unorder(np.ascontiguousarray(yT.T))
    return out
